# revision 2
# baseline (speedup 1.0000x reference)
"""CARC attention processor kernel v2 for 8 Trainium2 NeuronCores.

Reference computation (B=1, L=4096, C=640, H=10, D=64):
    q/k/v = hidden @ Wq/Wk/Wv, split into 10 heads of 64
    k_cat = [k, 0.42*K_bg], v_cat = [v, 0.42*V_bg]   (key length 8192)
    out   = softmax(q k_cat^T / 8) v_cat, heads merged, @ Wo + bo

Sharding: 4 query-blocks x 2 head-groups.  Core (hg, qb) computes heads
[5*hg, 5*hg+5) for queries [1024*qb, 1024*qb+1024) and emits the PARTIAL
output rows (sum over its 5 heads only, bias folded into group 0); the
host adds the two groups' partials per query block.  This halves the
replicated k/v projection work versus 8-way query sharding.

Engine plan (per core, ScalarE-bound design):
  - ScalarE: 5*1024*8192 = 42M exps in N=1536 chunks (3 PSUM banks) from
    a 2-deep score ring; the softmax denominator rides as a ones-column
    appended to V, the output bias as a ones-row appended to ctx.  A
    softmax-invariant bias of -2 keeps probs in hardware-friendly range.
  - TensorE: QK streams (K=64, output-port-bound).  PV runs in fp8e4
    with DoubleRow perf mode (2 key tiles per matmul) for the BACKGROUND
    source, whose probs max out near e^1.3 (well under fp8e4's 448); the
    SELF source has scaled scores reaching ~9.4 so its probs (up to
    ~e^7.4) stay bf16.  Projections are woven between attention chunks
    so the PE never blocks ScalarE.
  - VectorE: input casts, PSUM->SBUF copies, reciprocal normalize.
PSUM: 6 banks score ring (2 x [128,3,512]) + 1 bank ctx + 1 bank misc.
Per (head, query-chunk) the background source runs first: bg K/V need
only DMA+cast, which buys time for the woven self K/V projections.
"""

import numpy as np

import concourse.bass as bass
import concourse.mybir as mybir
import concourse.tile as tile

F32 = mybir.dt.float32
BF16 = mybir.dt.bfloat16
FP8 = mybir.dt.float8e4
AF = mybir.ActivationFunctionType
DR = mybir.MatmulPerfMode.DoubleRow

# Problem constants (hardcoded per contract)
B, L, C = 1, 4096, 640
H, D = 10, 64
ALPHA = 0.42
N_CORES = 8
SCALE = 1.0 / np.sqrt(D)  # 0.125

NQB = 4          # query blocks
NHG = 2          # head groups
HC = H // NHG    # 5 heads per core
Q = L // NQB     # 1024 queries per core
NPAIR = (HC + 1) // 2   # 3 partition-pair groups (h0h1, h2h3, h4-)
N_CC = C // 128  # 5 contraction chunks
NKT_SRC = L // 128   # 32 key tiles per source
VSTRIDE = 80     # per-head column stride, fp8 bg V arena (64 v + 1 ones + pad)
VSB = 66         # per-head column stride, bf16 self V arena (64 v + 1 ones + pad)

# (start_tile, n_tiles) chunks covering the 32 key tiles of one source
CHUNKS = [(3 * i, 3) for i in range(10)] + [(30, 2)]


def emit(nc: bass.Bass):
    hT = nc.declare_dram_parameter("hT", [C, L], F32, isOutput=False)
    hqT = nc.declare_dram_parameter("hqT", [C, Q], F32, isOutput=False)
    kbgT = nc.declare_dram_parameter("KbgT", [HC, D, L], F32, isOutput=False)
    vbg = nc.declare_dram_parameter("Vbg", [HC, L, D], F32, isOutput=False)
    wq = nc.declare_dram_parameter("Wq", [C, HC * D], F32, isOutput=False)
    wk = nc.declare_dram_parameter("Wk", [C, HC * D], F32, isOutput=False)
    wv = nc.declare_dram_parameter("Wv", [C, HC * D], F32, isOutput=False)
    wob = nc.declare_dram_parameter("WoB", [HC, D + 1, C], F32, isOutput=False)
    out = nc.declare_dram_parameter("out", [Q, C], F32, isOutput=True)

    with tile.TileContext(nc) as tc:
        with (
            tc.tile_pool(name="singles", bufs=1) as singles,
            tc.tile_pool(name="stage", bufs=2) as stage,
            tc.tile_pool(name="bgstage", bufs=2) as bgstage,
            tc.tile_pool(name="probs", bufs=3) as probs_pool,
            tc.tile_pool(name="fin", bufs=2) as fin_pool,
            tc.tile_pool(name="outsb", bufs=2) as outsb_pool,
            tc.tile_pool(name="ps_sc", bufs=2, space="PSUM") as ps_sc,
            tc.tile_pool(name="ps_ctx", bufs=1, space="PSUM") as ps_ctx,
            tc.tile_pool(name="ps_misc", bufs=1, space="PSUM") as ps_misc,
        ):
            # ---- persistent SBUF tensors ----
            hT_bf = singles.tile([128, N_CC, L], BF16, tag="hT_bf")
            hq_bf = singles.tile([128, N_CC, Q], BF16, tag="hq_bf")
            wq_bf = singles.tile([128, N_CC, HC * D], BF16, tag="wq_bf")
            wk_bf = singles.tile([128, N_CC, HC * D], BF16, tag="wk_bf")
            wv_bf = singles.tile([128, N_CC, HC * D], BF16, tag="wv_bf")
            wob_bf = singles.tile([D + 1, HC, C], BF16, tag="wob_bf")
            qT_bf = singles.tile([128, NPAIR, Q], BF16, tag="qT_bf")
            kT_bf = singles.tile([128, NPAIR, 2 * L], BF16, tag="kT_bf")
            varena = singles.tile([128, NKT_SRC, HC * VSTRIDE], FP8, tag="varena")
            vself = singles.tile([128, NKT_SRC, HC * VSB], BF16, tag="vself")
            ctxT = singles.tile([D + 1, HC, Q], BF16, tag="ctxT")
            ones65 = singles.tile([D + 1, D], BF16, tag="ones65")
            nc.vector.memset(ones65, 1.0)
            bneg = singles.tile([128, 1], F32, tag="bneg")
            nc.vector.memset(bneg, -2.0)
            for h in range(HC):
                nc.vector.memset(
                    varena[:, :, VSTRIDE * h + D : VSTRIDE * h + D + 1], 1.0
                )
                nc.vector.memset(
                    vself[:, :, VSB * h + D : VSB * h + D + 1], 1.0
                )

            # ---- weights + query-block hidden: load + cast ----
            for w_dram, w_sb in ((wq, wq_bf), (wk, wk_bf), (wv, wv_bf)):
                for half in range(2):
                    n0 = 160 * half
                    st = stage.tile([128, N_CC, 160], F32, tag="stage")
                    nc.sync.dma_start(
                        out=st,
                        in_=w_dram.rearrange("(i p) n -> p i n", p=128)[
                            :, :, n0 : n0 + 160
                        ],
                    )
                    nc.vector.tensor_copy(out=w_sb[:, :, n0 : n0 + 160], in_=st)
            for quar in range(4):
                n0 = 160 * quar
                st = stage.tile([D + 1, HC, 160], F32, tag="stage")
                nc.sync.dma_start(
                    out=st,
                    in_=wob[:, :, n0 : n0 + 160].rearrange("h p n -> p h n"),
                )
                nc.vector.tensor_copy(out=wob_bf[:, :, n0 : n0 + 160], in_=st)
            for s in range(4):
                st = stage.tile([128, N_CC, 256], F32, tag="stage")
                for i in range(N_CC):
                    nc.sync.dma_start(
                        out=st[:, i, :],
                        in_=hqT[128 * i : 128 * (i + 1), 256 * s : 256 * (s + 1)],
                    )
                nc.vector.tensor_copy(
                    out=hq_bf[:, :, 256 * s : 256 * (s + 1)], in_=st
                )

            # ---- q projection ----
            for qs in range(2):
                for g in range(NPAIR):
                    M = min(128, HC * D - 128 * g)
                    ps = ps_misc.tile([128, 512], F32, tag="mi", name=f"qp{qs}{g}")
                    for i in range(N_CC):
                        nc.tensor.matmul(
                            ps[0:M, :],
                            lhsT=wq_bf[:, i, 128 * g : 128 * g + M],
                            rhs=hq_bf[:, i, 512 * qs : 512 * (qs + 1)],
                            start=(i == 0),
                            stop=(i == N_CC - 1),
                        )
                    nc.vector.tensor_copy(
                        out=qT_bf[0:M, g, 512 * qs : 512 * (qs + 1)],
                        in_=ps[0:M, :],
                    )

            # ---- first background K/V slabs (gate the first attention pass)
            def bgk_item(g, sl):
                st = bgstage.tile([128, 1024], F32, tag="kbg_st", name=f"bk{g}{sl}")
                for half in range(2):
                    h = 2 * g + half
                    if h >= HC:
                        continue
                    nc.sync.dma_start(
                        out=st[64 * half : 64 * half + 64, :],
                        in_=kbgT[h, :, 1024 * sl : 1024 * (sl + 1)],
                    )
                M = 128 if 2 * g + 1 < HC else 64
                nc.vector.tensor_copy(
                    out=kT_bf[0:M, g, L + 1024 * sl : L + 1024 * (sl + 1)],
                    in_=st[0:M, :],
                )

            def bgv_item(h, sl):
                st = bgstage.tile([128, 8, D], F32, tag="vbg_st", name=f"bv{h}{sl}")
                nc.sync.dma_start(
                    out=st,
                    in_=vbg[h, 1024 * sl : 1024 * (sl + 1), :].rearrange(
                        "(kt q) d -> q kt d", q=128
                    ),
                )
                nc.vector.tensor_scalar_mul(
                    varena[
                        :,
                        8 * sl : 8 * (sl + 1),
                        VSTRIDE * h : VSTRIDE * h + D,
                    ],
                    st,
                    ALPHA,
                )

            bgk_item(0, 0)
            bgv_item(0, 0)
            bgv_item(1, 0)

            # ---- full hidden (transposed): DMA + cast, 16 slabs ----
            for s in range(16):
                st = stage.tile([128, N_CC, 256], F32, tag="stage")
                for i in range(N_CC):
                    nc.sync.dma_start(
                        out=st[:, i, :],
                        in_=hT[128 * i : 128 * (i + 1), 256 * s : 256 * (s + 1)],
                    )
                nc.vector.tensor_copy(
                    out=hT_bf[:, :, 256 * s : 256 * (s + 1)], in_=st
                )

            # ---- self K/V projection items (woven into attention) ----
            def kproj_item(g, s):
                M = min(128, HC * D - 128 * g)
                ps = ps_misc.tile([128, 512], F32, tag="mi", name=f"kp{g}{s}")
                for i in range(N_CC):
                    nc.tensor.matmul(
                        ps[0:M, :],
                        lhsT=wk_bf[:, i, 128 * g : 128 * g + M],
                        rhs=hT_bf[:, i, 512 * s : 512 * (s + 1)],
                        start=(i == 0),
                        stop=(i == N_CC - 1),
                    )
                nc.vector.tensor_copy(
                    out=kT_bf[0:M, g, 512 * s : 512 * (s + 1)],
                    in_=ps[0:M, :],
                )

            def vproj_item(kt):
                ps = ps_misc.tile([128, 512], F32, tag="mi", name=f"vp{kt}")
                for i in range(N_CC):
                    nc.tensor.matmul(
                        ps[:, 0 : HC * D],
                        lhsT=hT_bf[:, i, 128 * kt : 128 * (kt + 1)],
                        rhs=wv_bf[:, i, :],
                        start=(i == 0),
                        stop=(i == N_CC - 1),
                    )
                nc.vector.tensor_copy(
                    out=vself[:, kt, :].rearrange("p (h e) -> p h e", e=VSB)[
                        :, :, 0:D
                    ],
                    in_=ps[:, 0 : HC * D].rearrange("p (h d) -> p h d", d=D),
                )

            def outproj_item(qt):
                for n0, nw in ((0, 512), (512, 128)):
                    ps = ps_misc.tile([128, 512], F32, tag="mi", name=f"op{qt}{n0}")
                    for h in range(HC):
                        nc.tensor.matmul(
                            ps[:, 0:nw],
                            lhsT=ctxT[:, h, 128 * qt : 128 * (qt + 1)],
                            rhs=wob_bf[:, h, n0 : n0 + nw],
                            start=(h == 0),
                            stop=(h == HC - 1),
                        )
                    o_sb = outsb_pool.tile(
                        [128, 512], F32, tag="o_sb", name=f"ob{qt}{n0}"
                    )
                    nc.vector.tensor_copy(out=o_sb[:, 0:nw], in_=ps[:, 0:nw])
                    nc.sync.dma_start(
                        out=out[128 * qt : 128 * (qt + 1), n0 : n0 + nw],
                        in_=o_sb[:, 0:nw],
                    )

            # progress counters for dependency-forced drains
            kdone = [1, 0, 0]
            bgkdone = [1, 0, 0]
            vdone = [0]
            bgvdone = [1, 1] + [0] * (HC - 2)

            def emit_bg(item):
                kind = item[0]
                if kind == "k":
                    kproj_item(item[1], item[2])
                    kdone[item[1]] = max(kdone[item[1]], item[2] + 1)
                elif kind == "v":
                    vproj_item(item[1])
                    vdone[0] = max(vdone[0], item[1] + 1)
                elif kind == "bk":
                    bgk_item(item[1], item[2])
                    bgkdone[item[1]] = max(bgkdone[item[1]], item[2] + 1)
                elif kind == "bv":
                    bgv_item(item[1], item[2])
                    bgvdone[item[1]] = max(bgvdone[item[1]], item[2] + 1)
                elif kind == "op":
                    outproj_item(item[1])

            kproj_item(0, 0)
            kdone[0] = 1
            for kt in range(6):
                vproj_item(kt)
            vdone[0] = 6
            vq = list(range(6, NKT_SRC))

            bgq = []
            for sl in range(1, 4):
                bgq += [("bk", 0, sl), ("bv", 0, sl), ("bv", 1, sl)]
            bgq += [("k", 0, s) for s in range(1, 8)]
            for sl in range(4):
                bgq += [("bk", 1, sl), ("bv", 2, sl), ("bv", 3, sl)]
            bgq += [("k", 1, s) for s in range(8)]
            for sl in range(4):
                bgq += [("bk", 2, sl), ("bv", 4, sl)]
            bgq += [("k", 2, s) for s in range(8)]

            def drain_until(pred):
                while not pred():
                    assert bgq, "bg queue exhausted before dependency met"
                    emit_bg(bgq.pop(0))

            # ---- attention: per query-chunk, per head; bg source first ----
            chunk_idx = 0
            for qc in range(2):
                for h in range(HC):
                    g, half = h // 2, h % 2
                    p0, p1 = 64 * half, 64 * half + 64
                    ctx = ps_ctx.tile(
                        [D + 1, 512], F32, tag="ctx", name=f"ctx{qc}{h}"
                    )
                    first_mm = True
                    for src in (1, 0):
                        e_scale = SCALE if src == 0 else SCALE * ALPHA
                        for start_t, nt in CHUNKS:
                            hi_tile = start_t + nt
                            if src == 0:
                                drain_until(
                                    lambda: kdone[g]
                                    >= min((128 * hi_tile + 511) // 512, 8)
                                )
                                while vq and vq[0] < hi_tile:
                                    emit_bg(("v", vq.pop(0)))
                                drain_until(lambda: vdone[0] >= hi_tile)
                            else:
                                need = min((hi_tile + 7) // 8, 4)
                                drain_until(lambda: bgkdone[g] >= need)
                                drain_until(lambda: bgvdone[h] >= need)

                            scs = ps_sc.tile(
                                [128, 3, 512],
                                F32,
                                tag="sc",
                                name=f"sc{qc}{h}{src}{start_t}",
                            )
                            for j in range(nt):
                                kcol = L * src + 128 * (start_t + j)
                                nc.tensor.matmul(
                                    scs[:, j, :],
                                    lhsT=kT_bf[p0:p1, g, kcol : kcol + 128],
                                    rhs=qT_bf[p0:p1, g, 512 * qc : 512 * (qc + 1)],
                                    start=True,
                                    stop=True,
                                )
                            pr = probs_pool.tile(
                                [128, 3, 512],
                                FP8 if src == 1 else BF16,
                                tag="pr8" if src == 1 else "prb",
                                name=f"pr{qc}{h}{src}{start_t}",
                            )
                            nc.scalar.activation(
                                pr[:, 0:nt, :], scs[:, 0:nt, :], AF.Exp,
                                scale=e_scale, bias=bneg,
                            )
                            last_chunk = src == 0 and start_t == 30
                            if src == 1:
                                # bg PV: fp8 DoubleRow pair (+ single tile)
                                nc.tensor.matmul(
                                    ctx,
                                    lhsT=varena[
                                        :, start_t : start_t + 2,
                                        VSTRIDE * h : VSTRIDE * h + D + 1,
                                    ],
                                    rhs=pr[:, 0:2, :],
                                    perf_mode=DR,
                                    start=first_mm,
                                    stop=False,
                                )
                                first_mm = False
                                if nt == 3:
                                    nc.tensor.matmul(
                                        ctx,
                                        lhsT=varena[
                                            :, start_t + 2,
                                            VSTRIDE * h : VSTRIDE * h + D + 1,
                                        ],
                                        rhs=pr[:, 2, :],
                                        start=False,
                                        stop=False,
                                    )
                            else:
                                # self PV: bf16 singles (probs reach ~e^7.4)
                                for j in range(nt):
                                    nc.tensor.matmul(
                                        ctx,
                                        lhsT=vself[
                                            :, start_t + j,
                                            VSB * h : VSB * h + D + 1,
                                        ],
                                        rhs=pr[:, j, :],
                                        start=first_mm,
                                        stop=(last_chunk and j == nt - 1),
                                    )
                                    first_mm = False
                            # paced background weave
                            if qc == 0 and h == 0 and src == 1:
                                for _ in range(2):
                                    if vq:
                                        emit_bg(("v", vq.pop(0)))
                            elif bgq and chunk_idx % 2 == 0:
                                emit_bg(bgq.pop(0))
                            chunk_idx += 1

                    # ---- normalize head h, query chunk qc ----
                    dn = fin_pool.tile(
                        [D + 1, 512], BF16, tag="dn", name=f"dn{qc}{h}"
                    )
                    nc.vector.tensor_copy(
                        out=dn[D : D + 1, :], in_=ctx[D : D + 1, :]
                    )
                    bc = ps_misc.tile([128, 512], F32, tag="mi", name=f"bc{qc}{h}")
                    nc.tensor.matmul(
                        bc[0:D, :],
                        lhsT=ones65[D : D + 1, :],
                        rhs=dn[D : D + 1, :],
                        start=True,
                        stop=True,
                        tile_position=(D, 0),
                    )
                    rec = fin_pool.tile([D, 512], BF16, tag="rec", name=f"rc{qc}{h}")
                    with nc.allow_low_precision(
                        reason="softmax denom reciprocal; 0.4% bf16 noise ok"
                    ):
                        nc.vector.reciprocal(rec, bc[0:D, :])
                    nc.vector.tensor_mul(
                        ctxT[0:D, h, 512 * qc : 512 * (qc + 1)],
                        ctx[0:D, :],
                        rec,
                    )
                    nc.vector.memset(
                        ctxT[D : D + 1, h, 512 * qc : 512 * (qc + 1)], 1.0
                    )

                # output projection: weave qc0's into qc1, emit qc1's inline
                if qc == 0:
                    bgq[0:0] = [("op", qt) for qt in range(4)]
                else:
                    while bgq:
                        emit_bg(bgq.pop(0))
                    for qt in range(4, 8):
                        outproj_item(qt)

            assert not bgq and not vq, (len(bgq), len(vq))
    return nc


def split_waits(nc, limit=1):
    """This container's walrus rejects >limit sync waits per instruction;
    hoist excess waits onto standalone EventSemaphore instructions."""
    cnt = 0
    for f in nc.m.functions:
        for bb in f.blocks:
            fixed = []
            for inst in bb.instructions:
                si = inst.sync_info
                if si is not None and len(si.on_wait) > limit:
                    waits = list(si.on_wait)
                    extra, keep = waits[:-limit], waits[-limit:]
                    for w in extra:
                        cnt += 1
                        ev = mybir.InstEventSemaphore(
                            name=f"I-waitsplit-{cnt}", ins=[], outs=[]
                        )
                        ev.engine = inst.engine
                        ev.sync_info = mybir.SyncInfo(on_wait=[w], on_update=[])
                        nc.register_instruction(ev)
                        fixed.append(ev)
                    si.on_wait = keep
                fixed.append(inst)
            bb.instructions[:] = fixed
    return cnt


def build_bass():
    nc = bass.Bass()
    emit(nc)
    split_waits(nc)
    return nc


def shard_of_core(c):
    """core id -> (head group, query block)."""
    return c // NQB, c % NQB


def make_in_maps(hidden_states, K_bg, V_bg, Wq, Wk, Wv, Wo, bo):
    hT = np.ascontiguousarray(np.asarray(hidden_states, np.float32)[0].T)
    KbgT = np.ascontiguousarray(np.asarray(K_bg, np.float32).transpose(0, 2, 1))
    Vbg = np.ascontiguousarray(np.asarray(V_bg, np.float32))
    Wq = np.asarray(Wq, np.float32)
    Wk = np.asarray(Wk, np.float32)
    Wv = np.asarray(Wv, np.float32)
    Wo = np.asarray(Wo, np.float32)
    bo = np.asarray(bo, np.float32)

    per_hg = []
    for hg in range(NHG):
        cols = slice(HC * D * hg, HC * D * (hg + 1))
        wob5 = np.zeros((HC, D + 1, C), np.float32)
        wob5[:, :D, :] = Wo[cols].reshape(HC, D, C)
        if hg == 0:
            wob5[0, D, :] = bo
        per_hg.append(
            {
                "KbgT": np.ascontiguousarray(KbgT[HC * hg : HC * (hg + 1)]),
                "Vbg": np.ascontiguousarray(Vbg[HC * hg : HC * (hg + 1)]),
                "Wq": np.ascontiguousarray(Wq[:, cols]),
                "Wk": np.ascontiguousarray(Wk[:, cols]),
                "Wv": np.ascontiguousarray(Wv[:, cols]),
                "WoB": wob5,
            }
        )

    maps = []
    for c in range(N_CORES):
        hg, qb = shard_of_core(c)
        maps.append(
            dict(
                per_hg[hg],
                hT=hT,
                hqT=np.ascontiguousarray(hT[:, Q * qb : Q * (qb + 1)]),
            )
        )
    return maps


_NC_CACHE = {}


def assemble(results):
    """Combine per-core partial outputs into the full [B, L, C] output."""
    out = np.zeros((L, C), np.float32)
    for c in range(N_CORES):
        hg, qb = shard_of_core(c)
        out[Q * qb : Q * (qb + 1)] += results[c]["out"]
    return out.reshape(B, L, C)


def kernel(hidden_states, K_bg, V_bg, Wq, Wk, Wv, Wo, bo):
    if "nc" not in _NC_CACHE:
        _NC_CACHE["nc"] = build_bass()
    nc = _NC_CACHE["nc"]
    in_maps = make_in_maps(hidden_states, K_bg, V_bg, Wq, Wk, Wv, Wo, bo)
    from concourse import bass2jax

    results = bass2jax.run_bass_via_pjrt(nc, in_maps, n_cores=N_CORES)
    return assemble(results)


# revision 3
# speedup vs baseline: 1.1247x; 1.1247x over previous
"""CARC attention kernel v3 for 8 Trainium2 NeuronCores.

Sharding: 4 query-blocks x 2 head-groups (core = hg*4 + qb).  Each core
computes its 5 heads for its 1024 queries and emits a PARTIAL output;
the host adds the two head-groups' partials per query block.

Per-core hT is pre-ROTATED on the host so this core's query block is
always columns 0-1023 (self-attention is key-order invariant), making
the compiled module identical across cores and dropping the separate
query staging buffer.

Phase split (keeps the PE HAM clock-gate warm and avoids FIFO stalls):
  Phase A: background-source attention for all (head, query-chunk)
    passes.  Needs only the small bg K/V DMAs up front.  The hidden
    DMA+cast slabs and ALL self K/V projections weave through phase A's
    PE slack.  Partial ctx (with its denominator row) parks in SBUF
    as bf16.  PV runs fp8e4 DoubleRow (bg probs fit fp8 range).
  Phase B: self-source attention, everything resident; probs bf16
    (self scores reach ~9.4 so probs hit ~e^7.4).  Normalize adds the
    phase-A partial back in, then out-projection weaves into later
    chunks.
PSUM: 6 banks score ring (2 x [128,3,512]) + 1 bank ctx + 1 bank misc.
exp bias -2 (softmax-invariant) keeps fp8 probs in range.
"""

import numpy as np

import concourse.bass as bass
import concourse.mybir as mybir
import concourse.tile as tile

F32 = mybir.dt.float32
BF16 = mybir.dt.bfloat16
FP8 = mybir.dt.float8e4
AF = mybir.ActivationFunctionType
DR = mybir.MatmulPerfMode.DoubleRow

B, L, C = 1, 4096, 640
H, D = 10, 64
ALPHA = 0.42
N_CORES = 8
SCALE = 1.0 / np.sqrt(D)

NQB = 4
NHG = 2
HC = H // NHG
Q = L // NQB
NPAIR = (HC + 1) // 2
N_CC = C // 128
NKT_SRC = L // 128
VSTRIDE = 80  # fp8 bg V arena per-head stride (64 v + 1 ones + pad to 16B)
VSB = 66      # bf16 self V arena per-head stride (64 v + 1 ones + pad)

CHUNKS = [(3 * i, 3) for i in range(10)] + [(30, 2)]
KPROJ_GATE = 28  # earliest phase-A chunk index for self K/V projection weave


def emit(nc: bass.Bass):
    hT = nc.declare_dram_parameter("hT", [C, L], F32, isOutput=False)
    kbgT = nc.declare_dram_parameter("KbgT", [HC, D, L], F32, isOutput=False)
    vbg = nc.declare_dram_parameter("Vbg", [HC, L, D], F32, isOutput=False)
    wq = nc.declare_dram_parameter("Wq", [C, HC * D], F32, isOutput=False)
    wk = nc.declare_dram_parameter("Wk", [C, HC * D], F32, isOutput=False)
    wv = nc.declare_dram_parameter("Wv", [C, HC * D], F32, isOutput=False)
    wob = nc.declare_dram_parameter("WoB", [HC, D + 1, C], F32, isOutput=False)
    out = nc.declare_dram_parameter("out", [Q, C], F32, isOutput=True)

    with tile.TileContext(nc) as tc:
        with (
            tc.tile_pool(name="singles", bufs=1) as singles,
            tc.tile_pool(name="stage", bufs=2) as stage,
            tc.tile_pool(name="bgstage", bufs=2) as bgstage,
            tc.tile_pool(name="probs", bufs=3) as probs_pool,
            tc.tile_pool(name="fin", bufs=2) as fin_pool,
            tc.tile_pool(name="outsb", bufs=2) as outsb_pool,
            tc.tile_pool(name="ps_sc", bufs=2, space="PSUM") as ps_sc,
            tc.tile_pool(name="ps_ctx", bufs=1, space="PSUM") as ps_ctx,
            tc.tile_pool(name="ps_misc", bufs=1, space="PSUM") as ps_misc,
        ):
            hT_bf = singles.tile([128, N_CC, L], BF16, tag="hT_bf")
            wq_bf = singles.tile([128, N_CC, HC * D], BF16, tag="wq_bf")
            wk_bf = singles.tile([128, N_CC, HC * D], BF16, tag="wk_bf")
            wv_bf = singles.tile([128, N_CC, HC * D], BF16, tag="wv_bf")
            wob_bf = singles.tile([D + 1, HC, C], BF16, tag="wob_bf")
            qT_bf = singles.tile([128, NPAIR, Q], BF16, tag="qT_bf")
            kT_bf = singles.tile([128, NPAIR, 2 * L], BF16, tag="kT_bf")
            varena = singles.tile([128, NKT_SRC, HC * VSTRIDE], FP8, tag="va")
            vself = singles.tile([128, NKT_SRC, HC * VSB], BF16, tag="vs")
            ctxT = singles.tile([D + 1, HC, Q], BF16, tag="ctxT")
            ctxbg = singles.tile([D + 1, 2 * HC, 512], BF16, tag="ctxbg")
            ones65 = singles.tile([D + 1, D], BF16, tag="ones65")
            nc.vector.memset(ones65, 1.0)
            bneg = singles.tile([128, 1], F32, tag="bneg")
            nc.vector.memset(bneg, -2.0)
            for h in range(HC):
                nc.vector.memset(
                    varena[:, :, VSTRIDE * h + D : VSTRIDE * h + D + 1], 1.0
                )
                nc.vector.memset(
                    vself[:, :, VSB * h + D : VSB * h + D + 1], 1.0
                )

            # ---- item emitters -------------------------------------------
            def ht_item(s):
                st = stage.tile([128, N_CC, 256], F32, tag="stage")
                for i in range(N_CC):
                    nc.sync.dma_start(
                        out=st[:, i, :],
                        in_=hT[128 * i : 128 * (i + 1), 256 * s : 256 * (s + 1)],
                    )
                nc.vector.tensor_copy(
                    out=hT_bf[:, :, 256 * s : 256 * (s + 1)], in_=st
                )

            def qproj_item(qs, g):
                M = min(128, HC * D - 128 * g)
                ps = ps_misc.tile([128, 512], F32, tag="mi", name=f"qp{qs}{g}")
                for i in range(N_CC):
                    nc.tensor.matmul(
                        ps[0:M, :],
                        lhsT=wq_bf[:, i, 128 * g : 128 * g + M],
                        rhs=hT_bf[:, i, 512 * qs : 512 * (qs + 1)],
                        start=(i == 0),
                        stop=(i == N_CC - 1),
                    )
                nc.vector.tensor_copy(
                    out=qT_bf[0:M, g, 512 * qs : 512 * (qs + 1)], in_=ps[0:M, :]
                )

            def kproj_item(g, s):
                M = min(128, HC * D - 128 * g)
                ps = ps_misc.tile([128, 512], F32, tag="mi", name=f"kp{g}{s}")
                for i in range(N_CC):
                    nc.tensor.matmul(
                        ps[0:M, :],
                        lhsT=wk_bf[:, i, 128 * g : 128 * g + M],
                        rhs=hT_bf[:, i, 512 * s : 512 * (s + 1)],
                        start=(i == 0),
                        stop=(i == N_CC - 1),
                    )
                nc.vector.tensor_copy(
                    out=kT_bf[0:M, g, 512 * s : 512 * (s + 1)], in_=ps[0:M, :]
                )

            def vproj_item(kt):
                ps = ps_misc.tile([128, 512], F32, tag="mi", name=f"vp{kt}")
                for i in range(N_CC):
                    nc.tensor.matmul(
                        ps[:, 0 : HC * D],
                        lhsT=hT_bf[:, i, 128 * kt : 128 * (kt + 1)],
                        rhs=wv_bf[:, i, :],
                        start=(i == 0),
                        stop=(i == N_CC - 1),
                    )
                nc.vector.tensor_copy(
                    out=vself[:, kt, :].rearrange("p (h e) -> p h e", e=VSB)[
                        :, :, 0:D
                    ],
                    in_=ps[:, 0 : HC * D].rearrange("p (h d) -> p h d", d=D),
                )

            def bgk_item(g, sl):  # 512-key slabs, 8 per pair group
                st = bgstage.tile([128, 512], F32, tag="kbg", name=f"bk{g}{sl}")
                for half in range(2):
                    h = 2 * g + half
                    if h >= HC:
                        continue
                    nc.sync.dma_start(
                        out=st[64 * half : 64 * half + 64, :],
                        in_=kbgT[h, :, 512 * sl : 512 * (sl + 1)],
                    )
                M = 128 if 2 * g + 1 < HC else 64
                nc.vector.tensor_copy(
                    out=kT_bf[0:M, g, L + 512 * sl : L + 512 * (sl + 1)],
                    in_=st[0:M, :],
                )

            def bgv_item(h, sl):  # 1024-key slabs, 4 per head
                st = bgstage.tile([128, 8, D], F32, tag="vbg", name=f"bv{h}{sl}")
                nc.sync.dma_start(
                    out=st,
                    in_=vbg[h, 1024 * sl : 1024 * (sl + 1), :].rearrange(
                        "(kt q) d -> q kt d", q=128
                    ),
                )
                nc.vector.tensor_scalar_mul(
                    varena[:, 8 * sl : 8 * (sl + 1), VSTRIDE * h : VSTRIDE * h + D],
                    st,
                    ALPHA,
                )

            def outproj_item(qt):
                for n0, nw in ((0, 512), (512, 128)):
                    ps = ps_misc.tile([128, 512], F32, tag="mi", name=f"op{qt}{n0}")
                    for h in range(HC):
                        nc.tensor.matmul(
                            ps[:, 0:nw],
                            lhsT=ctxT[:, h, 128 * qt : 128 * (qt + 1)],
                            rhs=wob_bf[:, h, n0 : n0 + nw],
                            start=(h == 0),
                            stop=(h == HC - 1),
                        )
                    o_sb = outsb_pool.tile(
                        [128, 512], F32, tag="o_sb", name=f"ob{qt}{n0}"
                    )
                    nc.vector.tensor_copy(out=o_sb[:, 0:nw], in_=ps[:, 0:nw])
                    nc.sync.dma_start(
                        out=out[128 * qt : 128 * (qt + 1), n0 : n0 + nw],
                        in_=o_sb[:, 0:nw],
                    )

            # ---- startup -------------------------------------------------
            for half in range(2):
                n0 = 160 * half
                st = stage.tile([128, N_CC, 160], F32, tag="stage")
                nc.sync.dma_start(
                    out=st,
                    in_=wq.rearrange("(i p) n -> p i n", p=128)[:, :, n0 : n0 + 160],
                )
                nc.vector.tensor_copy(out=wq_bf[:, :, n0 : n0 + 160], in_=st)
            for s in range(4):
                ht_item(s)
            qproj_item(0, 0)
            qproj_item(1, 0)
            bgk_item(0, 0)
            bgv_item(0, 0)
            bgv_item(1, 0)
            for w_dram, w_sb in ((wk, wk_bf), (wv, wv_bf)):
                for half in range(2):
                    n0 = 160 * half
                    st = stage.tile([128, N_CC, 160], F32, tag="stage")
                    nc.sync.dma_start(
                        out=st,
                        in_=w_dram.rearrange("(i p) n -> p i n", p=128)[
                            :, :, n0 : n0 + 160
                        ],
                    )
                    nc.vector.tensor_copy(out=w_sb[:, :, n0 : n0 + 160], in_=st)
            for quar in range(4):
                n0 = 160 * quar
                st = stage.tile([D + 1, HC, 160], F32, tag="stage")
                nc.sync.dma_start(
                    out=st,
                    in_=wob[:, :, n0 : n0 + 160].rearrange("h p n -> p h n"),
                )
                nc.vector.tensor_copy(out=wob_bf[:, :, n0 : n0 + 160], in_=st)

            # ---- weave queues -------------------------------------------
            kdone = [0, 0, 0]
            vdone = [0]
            bgkdone = [1, 0, 0]
            bgvdone = [1, 1] + [0] * (HC - 2)
            qdone = {(0, 0), (1, 0)}
            htdone = [4]

            def emit_item(item):
                kind = item[0]
                if kind == "ht":
                    ht_item(item[1])
                    htdone[0] = max(htdone[0], item[1] + 1)
                elif kind == "qp":
                    qproj_item(item[1], item[2])
                    qdone.add((item[1], item[2]))
                elif kind == "k":
                    kproj_item(item[1], item[2])
                    kdone[item[1]] = max(kdone[item[1]], item[2] + 1)
                elif kind == "v":
                    vproj_item(item[1])
                    vdone[0] = max(vdone[0], item[1] + 1)
                elif kind == "bk":
                    bgk_item(item[1], item[2])
                    bgkdone[item[1]] = max(bgkdone[item[1]], item[2] + 1)
                elif kind == "bv":
                    bgv_item(item[1], item[2])
                    bgvdone[item[1]] = max(bgvdone[item[1]], item[2] + 1)
                elif kind == "op":
                    outproj_item(item[1])

            # interleave pair-0 bg slabs with hT slabs so the VectorE FIFO
            # order matches DMA arrival order (no head-of-line stalls)
            early_q = []
            hts = list(range(4, 16))
            g0_items = []
            for sl in range(1, 8):
                g0_items.append(("bk", 0, sl))
                if sl in (1, 3, 5):
                    g0_items.append(("bv", 0, (sl + 1) // 2))
                    g0_items.append(("bv", 1, (sl + 1) // 2))
            while g0_items or hts:
                if g0_items:
                    early_q.append(g0_items.pop(0))
                if hts:
                    early_q.append(("ht", hts.pop(0)))
            early_q += [("qp", 0, 1), ("qp", 1, 1), ("qp", 0, 2), ("qp", 1, 2)]
            for g in (1, 2):
                for sl in range(8):
                    early_q.append(("bk", g, sl))
                    if sl % 2 == 0:
                        for h in (2 * g, 2 * g + 1):
                            if h < HC:
                                early_q.append(("bv", h, sl // 2))
            late_q = [("k", 0, s) for s in range(8)]
            late_q += [("v", kt) for kt in range(NKT_SRC)]
            late_q += [("k", g, s) for g in (1, 2) for s in range(8)]

            def drain_early(pred):
                while not pred():
                    assert early_q, "early queue exhausted"
                    emit_item(early_q.pop(0))

            # ---- phase A: background attention --------------------------
            chunk_idx = 0
            for h in range(HC):
                g, half = h // 2, h % 2
                p0, p1 = 64 * half, 64 * half + 64
                drain_early(lambda: (0, g) in qdone and (1, g) in qdone)
                for qc in range(2):
                    ctx = ps_ctx.tile(
                        [D + 1, 512], F32, tag="ctx", name=f"cb{h}{qc}"
                    )
                    first_mm = True
                    for start_t, nt in CHUNKS:
                        hi = start_t + nt
                        drain_early(lambda: bgkdone[g] >= min((hi + 3) // 4, 8))
                        drain_early(lambda: bgvdone[h] >= min((hi + 7) // 8, 4))
                        scs = ps_sc.tile(
                            [128, 3, 512], F32, tag="sc", name=f"sA{h}{qc}{start_t}"
                        )
                        for j in range(nt):
                            kcol = L + 128 * (start_t + j)
                            nc.tensor.matmul(
                                scs[:, j, :],
                                lhsT=kT_bf[p0:p1, g, kcol : kcol + 128],
                                rhs=qT_bf[p0:p1, g, 512 * qc : 512 * (qc + 1)],
                                start=True,
                                stop=True,
                            )
                        pr = probs_pool.tile(
                            [128, 3, 512], FP8, tag="pr8", name=f"pA{h}{qc}{start_t}"
                        )
                        nc.scalar.activation(
                            pr[:, 0:nt, :], scs[:, 0:nt, :], AF.Exp,
                            scale=SCALE * ALPHA, bias=bneg,
                        )
                        nc.tensor.matmul(
                            ctx,
                            lhsT=varena[
                                :, start_t : start_t + 2,
                                VSTRIDE * h : VSTRIDE * h + D + 1,
                            ],
                            rhs=pr[:, 0:2, :],
                            perf_mode=DR,
                            start=first_mm,
                            stop=(nt == 2),
                        )
                        first_mm = False
                        if nt == 3:
                            nc.tensor.matmul(
                                ctx,
                                lhsT=varena[
                                    :, start_t + 2,
                                    VSTRIDE * h : VSTRIDE * h + D + 1,
                                ],
                                rhs=pr[:, 2, :],
                                start=False,
                                stop=False,
                            )
                        # weave: DMA/cast item every chunk; PE item when gated
                        if early_q:
                            emit_item(early_q.pop(0))
                        if late_q and chunk_idx >= KPROJ_GATE:
                            emit_item(late_q.pop(0))
                        chunk_idx += 1
                    nc.vector.tensor_copy(
                        out=ctxbg[:, 2 * h + qc, :], in_=ctx
                    )

            # ---- phase B: self attention --------------------------------
            while early_q:
                emit_item(early_q.pop(0))
            while late_q:
                emit_item(late_q.pop(0))
            weave_b = []
            for qc in range(2):
                for h in range(HC):
                    g, half = h // 2, h % 2
                    p0, p1 = 64 * half, 64 * half + 64
                    ctx = ps_ctx.tile(
                        [D + 1, 512], F32, tag="ctx", name=f"cs{qc}{h}"
                    )
                    first_mm = True
                    for start_t, nt in CHUNKS:
                        scs = ps_sc.tile(
                            [128, 3, 512], F32, tag="sc", name=f"sB{qc}{h}{start_t}"
                        )
                        for j in range(nt):
                            kcol = 128 * (start_t + j)
                            nc.tensor.matmul(
                                scs[:, j, :],
                                lhsT=kT_bf[p0:p1, g, kcol : kcol + 128],
                                rhs=qT_bf[p0:p1, g, 512 * qc : 512 * (qc + 1)],
                                start=True,
                                stop=True,
                            )
                        pr = probs_pool.tile(
                            [128, 3, 512], BF16, tag="prb", name=f"pB{qc}{h}{start_t}"
                        )
                        nc.scalar.activation(
                            pr[:, 0:nt, :], scs[:, 0:nt, :], AF.Exp,
                            scale=SCALE, bias=bneg,
                        )
                        last_chunk = start_t == 30
                        for j in range(nt):
                            nc.tensor.matmul(
                                ctx,
                                lhsT=vself[
                                    :, start_t + j, VSB * h : VSB * h + D + 1
                                ],
                                rhs=pr[:, j, :],
                                start=(first_mm and j == 0),
                                stop=(last_chunk and j == nt - 1),
                            )
                        first_mm = False
                        if weave_b:
                            emit_item(weave_b.pop(0))
                    # normalize: fold in phase-A partial, then 1/denom
                    nc.vector.tensor_add(
                        ctx, ctx, ctxbg[:, 2 * h + qc, :]
                    )
                    dn = fin_pool.tile([D + 1, 512], BF16, tag="dn", name=f"d{qc}{h}")
                    nc.vector.tensor_copy(out=dn[D : D + 1, :], in_=ctx[D : D + 1, :])
                    bc = ps_misc.tile([128, 512], F32, tag="mi", name=f"bc{qc}{h}")
                    nc.tensor.matmul(
                        bc[0:D, :],
                        lhsT=ones65[D : D + 1, :],
                        rhs=dn[D : D + 1, :],
                        start=True,
                        stop=True,
                        tile_position=(D, 0),
                    )
                    rec = fin_pool.tile([D, 512], BF16, tag="rec", name=f"r{qc}{h}")
                    with nc.allow_low_precision(
                        reason="softmax denom reciprocal; bf16 noise ok"
                    ):
                        nc.vector.reciprocal(rec, bc[0:D, :])
                    nc.vector.tensor_mul(
                        ctxT[0:D, h, 512 * qc : 512 * (qc + 1)], ctx[0:D, :], rec
                    )
                    nc.vector.memset(
                        ctxT[D : D + 1, h, 512 * qc : 512 * (qc + 1)], 1.0
                    )
                if qc == 0:
                    weave_b = [("op", qt) for qt in range(4)]
                else:
                    while weave_b:
                        emit_item(weave_b.pop(0))
                    for qt in range(4, 8):
                        outproj_item(qt)
    return nc


def split_waits(nc, limit=1):
    cnt = 0
    for f in nc.m.functions:
        for bb in f.blocks:
            fixed = []
            for inst in bb.instructions:
                si = inst.sync_info
                if si is not None and len(si.on_wait) > limit:
                    waits = list(si.on_wait)
                    extra, keep = waits[:-limit], waits[-limit:]
                    for w in extra:
                        cnt += 1
                        ev = mybir.InstEventSemaphore(
                            name=f"I-waitsplit-{cnt}", ins=[], outs=[]
                        )
                        ev.engine = inst.engine
                        ev.sync_info = mybir.SyncInfo(on_wait=[w], on_update=[])
                        nc.register_instruction(ev)
                        fixed.append(ev)
                    si.on_wait = keep
                fixed.append(inst)
            bb.instructions[:] = fixed
    return cnt


def build_bass():
    nc = bass.Bass()
    emit(nc)
    split_waits(nc)
    return nc


def shard_of_core(c):
    return c // NQB, c % NQB


def make_in_maps(hidden_states, K_bg, V_bg, Wq, Wk, Wv, Wo, bo):
    hT = np.ascontiguousarray(np.asarray(hidden_states, np.float32)[0].T)
    KbgT = np.ascontiguousarray(np.asarray(K_bg, np.float32).transpose(0, 2, 1))
    Vbg = np.ascontiguousarray(np.asarray(V_bg, np.float32))
    Wq = np.asarray(Wq, np.float32)
    Wk = np.asarray(Wk, np.float32)
    Wv = np.asarray(Wv, np.float32)
    Wo = np.asarray(Wo, np.float32)
    bo = np.asarray(bo, np.float32)

    per_hg = []
    for hg in range(NHG):
        cols = slice(HC * D * hg, HC * D * (hg + 1))
        wob5 = np.zeros((HC, D + 1, C), np.float32)
        wob5[:, :D, :] = Wo[cols].reshape(HC, D, C)
        if hg == 0:
            wob5[0, D, :] = bo
        per_hg.append(
            {
                "KbgT": np.ascontiguousarray(KbgT[HC * hg : HC * (hg + 1)]),
                "Vbg": np.ascontiguousarray(Vbg[HC * hg : HC * (hg + 1)]),
                "Wq": np.ascontiguousarray(Wq[:, cols]),
                "Wk": np.ascontiguousarray(Wk[:, cols]),
                "Wv": np.ascontiguousarray(Wv[:, cols]),
                "WoB": wob5,
            }
        )
    # per-qb rotated hT: query block at columns 0-1023
    hT_rot = [
        np.ascontiguousarray(np.concatenate([hT[:, Q * qb :], hT[:, : Q * qb]], 1))
        for qb in range(NQB)
    ]
    maps = []
    for c in range(N_CORES):
        hg, qb = shard_of_core(c)
        maps.append(dict(per_hg[hg], hT=hT_rot[qb]))
    return maps


_NC_CACHE = {}


def assemble(results):
    out = np.zeros((L, C), np.float32)
    for c in range(N_CORES):
        hg, qb = shard_of_core(c)
        out[Q * qb : Q * (qb + 1)] += results[c]["out"]
    return out.reshape(B, L, C)


def kernel(hidden_states, K_bg, V_bg, Wq, Wk, Wv, Wo, bo):
    if "nc" not in _NC_CACHE:
        _NC_CACHE["nc"] = build_bass()
    nc = _NC_CACHE["nc"]
    in_maps = make_in_maps(hidden_states, K_bg, V_bg, Wq, Wk, Wv, Wo, bo)
    from concourse import bass2jax

    results = bass2jax.run_bass_via_pjrt(nc, in_maps, n_cores=N_CORES)
    return assemble(results)
